# revision 24
# baseline (speedup 1.0000x reference)
"""MoIE transformer block — Bass/Tile kernel for 8 Trainium2 NeuronCores.

Contract: kernel(**inputs) takes FULL unsharded inputs (B=4, S=2048, D=1024,
fp32) and returns the FULL [4, 2048, 1024] fp32 output.

Sharding: data-parallel, 2 cores per batch (core c -> batch c//2, half c%2).
Each core owns 1024 query rows and recomputes LN + k/v branches over the full
2048-row sequence of its batch (weights replicated). The SPMD program is
identical on all cores; role differences (which half) are pure data:

  - the host feeds x[b]^T with COLUMNS PERMUTED so the core's own query
    tokens always sit in columns [0:1024),
  - causality is applied with a host-built 0/1 mask (multiplied into exp(s)),
    so the attention loop shape is uniform across cores.

Layout is feature-major ([d partitions, tokens free]) end to end:
  - LN stats via ones-vector matmuls (sum / sum-of-squares over partitions),
    rstd/offset broadcast back across partitions with a K=1 ones matmul.
  - branch matmuls: out[o_tile, n] = W^T_tile.T @ ln1_tile, bf16 in / fp32
    PSUM out; routing chain = Silu(comp+b) on ACT, (match/32 - cost) and
    relu on DVE, passthrough select via copy_predicated with mrl as the
    predicate (mrl != 0  <=>  rl > 0).
  - attention computes scores TRANSPOSED ([k partitions, q free], K
    stationary) so softmax needs no P-transpose: exp on ACT (constant -8
    bias for range safety; exact softmax is shift-invariant), 0/1 mask
    multiply, denominator via ones-matmul over partitions, P@V directly
    from the transposed exp tiles. Only V needs a real transpose (DMA xbar,
    SBUF->SBUF, 128x128 bf16 blocks), streamed chunk by chunk.
  - o-branch + residual in fp32, output stored feature-major; host
    transposes back.
"""

import contextlib
import os
import sys

import numpy as np

for _p in ("/opt/trn_rl_repo",):
    if _p not in sys.path and os.path.isdir(_p):
        sys.path.insert(0, _p)

import ml_dtypes  # noqa: E402

import concourse.bass as bass  # noqa: E402,F401
import concourse.mybir as mybir  # noqa: E402
import concourse.tile as tile  # noqa: E402
from concourse import bacc  # noqa: E402
from concourse.bass_utils import run_bass_kernel_spmd  # noqa: E402

B, S, D = 4, 2048, 1024
P = 128
DC = D // P          # 8 d-chunks
KB = S // P          # 16 key blocks
NQ = S // 2          # 1024 query tokens per core
EPS_LN = 1e-5
EPS_MAS = 1e-9
INV_SQRT_D = 1.0 / 32.0
EXP_SHIFT = -8.0     # constant softmax shift (exact softmax is shift-invariant)

F32 = mybir.dt.float32
BF16 = mybir.dt.bfloat16
OP = mybir.AluOpType
ACTF = mybir.ActivationFunctionType

BF = ml_dtypes.bfloat16

USE_COLLECTIVES = bool(int(os.environ.get("BASS_USE_CC", "1")))

_COMPILED = {}
LAST_EXEC_TIME_NS = None

WEIGHT_NAMES = [f"{n}_{w}" for n in "qkvo" for w in ("mu", "pr")]
# vecs rows: ln_g, ln_b, cost_q, cost_k, cost_v, cost_o, bias_q, bias_k, bias_v, bias_o
VROW = {"ln_g": 0, "ln_b": 1, "cost_q": 2, "cost_k": 3, "cost_v": 4,
        "cost_o": 5, "bias_q": 6, "bias_k": 7, "bias_v": 8, "bias_o": 9}


def _branch(nc, psum, sbuf, n, rhs_sb, wmu, wpr, negcost_ap, cost32_ap,
            bias_ap, pass_sb, out_cb):
    """Routed-expert branch, feature-major.

    rhs_sb/pass_sb: [P, DC, n] bf16.  wmu/wpr: [P, DC, D] bf16 (lhsT layout).
    negcost_ap/cost32_ap/bias_ap: [P, DC] fp32 per-partition scalar views
    (-cost, 32*cost, mu bias).
    out_cb(ot, j) -> (dst_ap [P,512], finish_fn|None): routed bf16 output
    chunk destination; finish_fn runs after the chunk is complete.

    out = comp*relu(rl) + pass*(rl<=0), rl = match/32 - cost:
      ACT: comp = Silu(cp + bias);  mrl = Relu(mp/32 - cost)  [fused scale+bias]
      DVE: passc = (mp <= 32cost)*pass; prod = comp*mrl; dst = prod + passc
    """
    nch = n // 512
    for ot in range(DC):
        for j in range(nch):
            cp = psum.tile([P, 512], F32, tag="mm")
            mp = psum.tile([P, 512], F32, tag="mm")
            for dc in range(DC):
                nc.tensor.matmul(cp, wmu[:, dc, ot * P:(ot + 1) * P],
                                 rhs_sb[:, dc, j * 512:(j + 1) * 512],
                                 start=(dc == 0), stop=(dc == DC - 1))
            for dc in range(DC):
                nc.tensor.matmul(mp, wpr[:, dc, ot * P:(ot + 1) * P],
                                 rhs_sb[:, dc, j * 512:(j + 1) * 512],
                                 start=(dc == 0), stop=(dc == DC - 1))
            comp = sbuf.tile([P, 512], BF16, tag="comp")
            nc.scalar.activation(comp, cp, ACTF.Silu, bias=bias_ap[:, ot:ot + 1])
            mrl = sbuf.tile([P, 512], BF16, tag="mrl")
            nc.scalar.activation(mrl, mp, ACTF.Relu,
                                 bias=negcost_ap[:, ot:ot + 1], scale=INV_SQRT_D)
            passc = sbuf.tile([P, 512], BF16, tag="passc")
            nc.vector.scalar_tensor_tensor(
                passc, mp, cost32_ap[:, ot:ot + 1],
                pass_sb[:, ot, j * 512:(j + 1) * 512], OP.is_le, OP.mult)
            prod = sbuf.tile([P, 512], BF16, tag="prod")
            nc.vector.tensor_mul(prod, comp, mrl)
            dst, finish = out_cb(ot, j)
            nc.vector.tensor_add(dst, prod, passc)
            if finish is not None:
                finish()


def _build_program():
    nc = bacc.Bacc("TRN2", target_bir_lowering=False, debug=False,
                   enable_asserts=False, num_devices=8)

    NKV = NQ if USE_COLLECTIVES else S   # kv tokens computed locally per core

    xT = nc.dram_tensor("xT", [D, S], BF16, kind="ExternalInput").ap()
    maskT = nc.dram_tensor("maskT", [S, NQ], BF16, kind="ExternalInput").ap()
    vecs = nc.dram_tensor("vecs", [10, D], F32, kind="ExternalInput").ap()
    w_dram = {n: nc.dram_tensor(f"w_{n}", [D, D], BF16, kind="ExternalInput").ap()
              for n in WEIGHT_NAMES}
    out = nc.dram_tensor("out_fm", [D, NQ], F32, kind="ExternalOutput").ap()
    attn_dram = nc.dram_tensor("attn_bounce", [D, NQ], BF16).ap()
    k_loc = nc.dram_tensor("k_loc", [D, NKV], BF16).ap()
    v_loc = nc.dram_tensor("v_loc", [D, NKV], BF16).ap()
    if USE_COLLECTIVES:
        k_gath = nc.dram_tensor("k_gath", [2, D, NQ], BF16).ap()
        v_gath = nc.dram_tensor("v_gath", [2, D, NQ], BF16).ap()

    xT3 = xT.rearrange("(dc p) n -> p dc n", p=P)
    maskT3 = maskT.rearrange("(kb p) q -> p kb q", p=P)
    vecs3 = vecs.rearrange("r (oc p) -> p r oc", p=P)
    out3 = out.rearrange("(dc p) n -> p dc n", p=P)
    attn3 = attn_dram.rearrange("(dc p) n -> p dc n", p=P)
    k_loc3 = k_loc.rearrange("(dc p) n -> p dc n", p=P)
    v_loc3 = v_loc.rearrange("(dc p) n -> p dc n", p=P)

    with tile.TileContext(nc) as tc, contextlib.ExitStack() as ctx:
        consts = ctx.enter_context(tc.tile_pool(name="consts", bufs=1))
        wpool = ctx.enter_context(tc.tile_pool(name="weights", bufs=4))
        stream = ctx.enter_context(tc.tile_pool(name="stream", bufs=3))
        psum = ctx.enter_context(tc.tile_pool(name="psum", bufs=6, space="PSUM"))
        psum1 = ctx.enter_context(tc.tile_pool(name="psum1", bufs=2, space="PSUM"))

        vec_sb = consts.tile([P, 10, DC], F32)
        nc.sync.dma_start(vec_sb, vecs3)
        ones_bf = consts.tile([P, 1], BF16)
        nc.vector.memset(ones_bf, 1.0)
        ones_row = consts.tile([1, P], F32)
        nc.vector.memset(ones_row, 1.0)
        eps_ap = consts.tile([P, 1], F32)
        nc.vector.memset(eps_ap, float(EPS_LN))
        esh_ap = consts.tile([P, 1], F32)
        nc.vector.memset(esh_ap, float(EXP_SHIFT))
        negcost = consts.tile([P, 4, DC], F32)
        cost32 = consts.tile([P, 4, DC], F32)
        for i, nme in enumerate("qkvo"):
            nc.vector.tensor_scalar_mul(negcost[:, i, :],
                                        vec_sb[:, VROW[f"cost_{nme}"], :], -1.0)
            nc.vector.tensor_scalar_mul(cost32[:, i, :],
                                        vec_sb[:, VROW[f"cost_{nme}"], :], 32.0)
        CIDX = {n: i for i, n in enumerate("qkvo")}

        def vrow(name):
            return vec_sb[:, VROW[name], :]

        def wload(name):
            t = wpool.tile([P, DC, D], BF16, tag="w")
            nc.sync.dma_start(t, w_dram[name].rearrange("(dc p) o -> p dc o", p=P))
            return t

        q_pool = ctx.enter_context(tc.tile_pool(name="q_pool", bufs=1))
        q_fm = q_pool.tile([P, DC, NQ], BF16, tag="q_fm")

        with tc.tile_pool(name="ln_scope", bufs=1) as lnp:
            ln1 = lnp.tile([P, DC, NKV], BF16, tag="ln1")

            # ------------- LayerNorm (feature-major, local tokens) -------------
            with tc.tile_pool(name="ln_bc", bufs=1) as lnbc, \
                 tc.tile_pool(name="ln_stream", bufs=16) as lns, \
                 tc.tile_pool(name="ln_stats", bufs=6) as lnst, \
                 tc.tile_pool(name="ln_sq", bufs=8) as lnsq:
                rstd_bc = lnbc.tile([P, NKV], BF16, tag="rstd_bc")
                boff_bc = lnbc.tile([P, NKV], BF16, tag="boff_bc")
                xts = {}
                for j in range(NKV // 512):
                    s_ps = psum1.tile([1, 512], F32, tag="vsum")
                    q_ps = psum1.tile([1, 512], F32, tag="vsum")
                    sqs = []
                    for dc in range(DC):
                        xt = lns.tile([P, 512], BF16, tag="xt")
                        nc.sync.dma_start(xt, xT3[:, dc, j * 512:(j + 1) * 512])
                        xts[(j, dc)] = xt
                        sq = lnsq.tile([P, 512], BF16, tag="sq")
                        nc.scalar.square(sq, xt)
                        sqs.append(sq)
                        nc.tensor.matmul(s_ps, ones_bf, xt,
                                         start=(dc == 0), stop=(dc == DC - 1))
                    for dc in range(DC):
                        nc.tensor.matmul(q_ps, ones_bf, sqs[dc],
                                         start=(dc == 0), stop=(dc == DC - 1))
                    mean = lnst.tile([1, 512], F32, tag="stat")
                    nc.vector.tensor_scalar_mul(mean, s_ps, 1.0 / D)
                    var = lnst.tile([1, 512], F32, tag="stat")
                    nc.vector.tensor_scalar_mul(var, q_ps, 1.0 / D)
                    m2 = lnst.tile([1, 512], F32, tag="stat")
                    nc.vector.tensor_mul(m2, mean, mean)
                    nc.vector.tensor_sub(var, var, m2)
                    nc.scalar.activation(var, var, ACTF.Sqrt, bias=eps_ap[:1])
                    rstd = lnst.tile([1, 512], F32, tag="stat")
                    nc.vector.reciprocal(rstd, var)
                    boff = lnst.tile([1, 512], F32, tag="stat")
                    nc.vector.tensor_mul(boff, mean, rstd)
                    nc.vector.tensor_scalar_mul(boff, boff, -1.0)
                    # broadcast across partitions via K=1 ones matmul
                    bc = psum1.tile([P, 512], F32, tag="vsum")
                    nc.tensor.matmul(bc, ones_row, rstd, start=True, stop=True)
                    nc.scalar.copy(rstd_bc[:, j * 512:(j + 1) * 512], bc)
                    bc2 = psum1.tile([P, 512], F32, tag="vsum")
                    nc.tensor.matmul(bc2, ones_row, boff, start=True, stop=True)
                    nc.scalar.copy(boff_bc[:, j * 512:(j + 1) * 512], bc2)
                # pass 2: apply from the resident pass-1 tiles, j-chunked
                # so the k-branch can start on chunk 0 early
                for j in range(NKV // 512):
                    sl = slice(j * 512, (j + 1) * 512)
                    for dc in range(DC):
                        dst = ln1[:, dc, sl]
                        nc.vector.tensor_mul(dst, xts[(j, dc)], rstd_bc[:, sl])
                        nc.vector.tensor_add(dst, dst, boff_bc[:, sl])
                        nc.vector.tensor_scalar(dst, dst,
                                                vrow("ln_g")[:, dc:dc + 1],
                                                vrow("ln_b")[:, dc:dc + 1],
                                                OP.mult, OP.add)

            # ------------- k / v / q branches -------------
            # k and v stream their routed output chunks straight to DRAM
            # (k_loc / v_loc), to be (all-)gathered + reloaded for attention.
            def dram_out(dram3):
                def cb(ot, j):
                    chunk = stream.tile([P, 512], BF16, tag="dchunk")

                    def finish():
                        nc.sync.dma_start(dram3[:, ot, j * 512:(j + 1) * 512],
                                          chunk)
                    return chunk, finish
                return cb

            wkm, wkp = wload("k_mu"), wload("k_pr")
            _branch(nc, psum, stream, NKV, ln1, wkm, wkp,
                    negcost[:, CIDX["k"], :], cost32[:, CIDX["k"], :],
                    vrow("bias_k"), ln1, dram_out(k_loc3))
            wvm, wvp = wload("v_mu"), wload("v_pr")
            _branch(nc, psum, stream, NKV, ln1, wvm, wvp,
                    negcost[:, CIDX["v"], :], cost32[:, CIDX["v"], :],
                    vrow("bias_v"), ln1, dram_out(v_loc3))
            if USE_COLLECTIVES:
                groups = [[0, 1], [2, 3], [4, 5], [6, 7]]
                nc.gpsimd.collective_compute(
                    "AllGather", mybir.AluOpType.bypass, replica_groups=groups,
                    ins=[k_loc[:]], outs=[k_gath[:]])
                nc.gpsimd.collective_compute(
                    "AllGather", mybir.AluOpType.bypass, replica_groups=groups,
                    ins=[v_loc[:]], outs=[v_gath[:]])
            wqm, wqp = wload("q_mu"), wload("q_pr")
            _branch(nc, psum, stream, NQ, ln1[:, :, :NQ], wqm, wqp,
                    negcost[:, CIDX["q"], :], cost32[:, CIDX["q"], :],
                    vrow("bias_q"), ln1[:, :, :NQ],
                    lambda ot, j: (q_fm[:, ot, j * 512:(j + 1) * 512], None))

        # ---------------- attention ----------------
        with tc.tile_pool(name="kv_pool", bufs=1) as kvp, \
             tc.tile_pool(name="attn_tmp", bufs=1) as at, \
             tc.tile_pool(name="mask_pool", bufs=1) as mkp, \
             tc.tile_pool(name="attn_sc", bufs=2) as asc, \
             tc.tile_pool(name="attn_st", bufs=3) as ast:
            k_fm = kvp.tile([P, DC, S], BF16, tag="k_fm")
            v_tm = kvp.tile([P, KB, D], BF16, tag="v_tm")
            if USE_COLLECTIVES:
                for g in range(2):
                    kg3 = k_gath[g].rearrange("(dc p) n -> p dc n", p=P)
                    for dc in range(DC):
                        nc.sync.dma_start(k_fm[:, dc, g * NQ:(g + 1) * NQ],
                                          kg3[:, dc, :])
                    for dt in range(DC):
                        eng = nc.scalar if dt % 2 else nc.sync
                        eng.dma_start(
                            v_tm[:, g * 8:(g + 1) * 8, dt * P:(dt + 1) * P],
                            v_gath[g, dt * P:(dt + 1) * P, :],
                            transpose=True)
            else:
                for dc in range(DC):
                    nc.sync.dma_start(k_fm[:, dc, :], k_loc3[:, dc, :])
                for dt in range(DC):
                    eng = nc.scalar if dt % 2 else nc.sync
                    eng.dma_start(
                        v_tm[:, :, dt * P:(dt + 1) * P],
                        v_loc[dt * P:(dt + 1) * P, :],
                        transpose=True)

            for qg in range(NQ // 512):
                q0 = qg * 512
                # With natural (gathered) key order, the highest query token
                # in this group is NQ+q0+511; later key blocks are masked on
                # BOTH core roles -> skip. (Invalid for the permuted no-CC
                # key order, where high columns hold earlier tokens.)
                nkb = KB
                mask_sb = mkp.tile([P, KB, 512], BF16, tag="mask")
                nc.sync.dma_start(mask_sb[:, :nkb, :], maskT3[:, :nkb, q0:q0 + 512])
                expT = at.tile([P, KB, 512], BF16, tag="expT")
                for kb in range(nkb):
                    sT = psum.tile([P, 512], F32, tag="mm")
                    for dc in range(DC):
                        nc.tensor.matmul(
                            sT, k_fm[:, dc, kb * P:(kb + 1) * P],
                            q_fm[:, dc, q0:q0 + 512],
                            start=(dc == 0), stop=(dc == DC - 1))
                    # exp(s/sqrt(D) - 8); masked cols zeroed after
                    nc.scalar.activation(expT[:, kb, :], sT, ACTF.Exp,
                                         bias=esh_ap, scale=INV_SQRT_D)
                    nc.vector.tensor_mul(expT[:, kb, :], expT[:, kb, :],
                                         mask_sb[:, kb, :])
                sum_ps = psum1.tile([1, 512], F32, tag="vsum")
                for kb in range(nkb):
                    nc.tensor.matmul(sum_ps, ones_bf, expT[:, kb, :],
                                     start=(kb == 0), stop=(kb == nkb - 1))
                rcp = asc.tile([1, 512], F32, tag="rcp")
                nc.vector.reciprocal(rcp, sum_ps)
                bc = psum1.tile([P, 512], F32, tag="vsum")
                nc.tensor.matmul(bc, ones_row, rcp, start=True, stop=True)
                rcpb = asc.tile([P, 512], F32, tag="rcpb")
                nc.scalar.copy(rcpb, bc)
                for dt in range(DC):
                    av = psum.tile([P, 512], F32, tag="mm")
                    for kb in range(nkb):
                        nc.tensor.matmul(av, v_tm[:, kb, dt * P:(dt + 1) * P],
                                         expT[:, kb, :],
                                         start=(kb == 0), stop=(kb == nkb - 1))
                    a_sb = ast.tile([P, 512], BF16, tag="a_sb")
                    nc.vector.tensor_mul(a_sb, av, rcpb)
                    nc.sync.dma_start(attn3[:, dt, q0:q0 + 512], a_sb)

        # ---------------- o branch + residual ----------------
        with tc.tile_pool(name="xq_pool", bufs=1) as xqp, \
             tc.tile_pool(name="o_stream", bufs=3) as ost:
            xq = xqp.tile([P, DC, NQ], BF16, tag="xq")
            nc.sync.dma_start(xq, xT3[:, :, :NQ])
            attn = xqp.tile([P, DC, NQ], BF16, tag="attn_sb")
            for j0 in range(NQ // 512):
                for dc in range(DC):
                    nc.sync.dma_start(attn[:, dc, j0 * 512:(j0 + 1) * 512],
                                      attn3[:, dc, j0 * 512:(j0 + 1) * 512])
            wom, wop = wload("o_mu"), wload("o_pr")

            nch = NQ // 512
            for j in range(nch):
                for ot in range(DC):
                    cp = psum.tile([P, 512], F32, tag="mm")
                    mp = psum.tile([P, 512], F32, tag="mm")
                    for dc in range(DC):
                        nc.tensor.matmul(cp, wom[:, dc, ot * P:(ot + 1) * P],
                                         attn[:, dc, j * 512:(j + 1) * 512],
                                         start=(dc == 0), stop=(dc == DC - 1))
                    for dc in range(DC):
                        nc.tensor.matmul(mp, wop[:, dc, ot * P:(ot + 1) * P],
                                         attn[:, dc, j * 512:(j + 1) * 512],
                                         start=(dc == 0), stop=(dc == DC - 1))
                    comp = stream.tile([P, 512], BF16, tag="comp")
                    nc.scalar.activation(comp, cp, ACTF.Silu,
                                         bias=vrow("bias_o")[:, ot:ot + 1])
                    mrl = stream.tile([P, 512], BF16, tag="mrl")
                    nc.scalar.activation(mrl, mp, ACTF.Relu,
                                         bias=negcost[:, CIDX["o"], ot:ot + 1],
                                         scale=INV_SQRT_D)
                    passc = ost.tile([P, 512], F32, tag="passcf")
                    nc.vector.scalar_tensor_tensor(
                        passc, mp, cost32[:, CIDX["o"], ot:ot + 1],
                        attn[:, ot, j * 512:(j + 1) * 512], OP.is_le, OP.mult)
                    prod = ost.tile([P, 512], F32, tag="prodf")
                    nc.vector.tensor_mul(prod, comp, mrl)
                    ot_f = ost.tile([P, 512], F32, tag="outf")
                    nc.vector.tensor_add(ot_f, prod, passc)
                    nc.vector.tensor_add(ot_f, ot_f,
                                         xq[:, ot, j * 512:(j + 1) * 512])
                    nc.sync.dma_start(out3[:, ot, j * 512:(j + 1) * 512], ot_f)

    nc.compile()
    return nc


def _get_program():
    if "nc" not in _COMPILED:
        _COMPILED["nc"] = _build_program()
    return _COMPILED["nc"]


def _prepare_inputs(x, ln_g, ln_b,
                    q_mu_w, q_mu_b, q_proto, q_gate,
                    k_mu_w, k_mu_b, k_proto, k_gate,
                    v_mu_w, v_mu_b, v_proto, v_gate,
                    o_mu_w, o_mu_b, o_proto, o_gate):
    x = np.asarray(x, dtype=np.float32)

    w_host = {}
    for n, (mu_w, proto) in zip("qkvo", [(q_mu_w, q_proto), (k_mu_w, k_proto),
                                         (v_mu_w, v_proto), (o_mu_w, o_proto)]):
        w_host[f"w_{n}_mu"] = np.ascontiguousarray(np.asarray(mu_w).T).astype(BF)
        w_host[f"w_{n}_pr"] = np.ascontiguousarray(np.asarray(proto).T).astype(BF)

    vecs = np.zeros((10, D), np.float32)
    vecs[VROW["ln_g"]] = np.asarray(ln_g)
    vecs[VROW["ln_b"]] = np.asarray(ln_b)
    for n, gate, bias in [("q", q_gate, q_mu_b), ("k", k_gate, k_mu_b),
                          ("v", v_gate, v_mu_b), ("o", o_gate, o_mu_b)]:
        g = np.asarray(gate, np.float32)
        vecs[VROW[f"cost_{n}"]] = g / (np.abs(g).max() + EPS_MAS)
        vecs[VROW[f"bias_{n}"]] = np.asarray(bias, np.float32)

    in_maps = []
    orders = []
    for c in range(8):
        b, h = c // 2, c % 2
        order = (np.arange(S) + h * NQ) % S
        orders.append((b, order))
        xTc = np.ascontiguousarray(x[b][order].T).astype(BF)
        key_tok = np.arange(S) if USE_COLLECTIVES else order
        maskT = (key_tok[:, None] <= order[None, :NQ]).astype(BF)
        m = dict(w_host)
        m["xT"] = xTc
        m["maskT"] = np.ascontiguousarray(maskT)
        m["vecs"] = vecs
        in_maps.append(m)
    return in_maps, orders


def _ensure_ntff_hook():
    """Install the axon NTFF profile hook if the image's antenv lacks it."""
    try:
        from antenv.axon_hooks import get_axon_ntff_profile_hook  # noqa: F401
        return
    except ImportError:
        pass
    try:
        import types

        from trn_agent_boot.trn_boot import _ntff_profile_via_ctypes
        hook = _ntff_profile_via_ctypes("/opt/axon/libaxon_pjrt.so")
        mod = types.ModuleType("antenv.axon_hooks")
        mod.get_axon_ntff_profile_hook = lambda: hook
        mod.set_axon_ntff_profile_hook = lambda h: None
        import antenv
        sys.modules["antenv.axon_hooks"] = mod
        antenv.axon_hooks = mod
    except Exception as e:  # degrade to untraced run
        print(f"ntff hook install failed: {e}", file=sys.stderr)


def kernel(**inputs):
    global LAST_EXEC_TIME_NS
    in_maps, orders = _prepare_inputs(**inputs)
    nc = _get_program()
    trace = bool(int(os.environ.get("BASS_KERNEL_TRACE", "0")))
    if trace:
        _ensure_ntff_hook()
    tdir = os.environ.get("BASS_KERNEL_TRACE_DIR") or None
    res = run_bass_kernel_spmd(nc, in_maps, list(range(8)), trace=trace,
                               tmpdir=tdir)
    LAST_EXEC_TIME_NS = res.exec_time_ns

    out = np.empty((B, S, D), np.float32)
    for c in range(8):
        b, order = orders[c]
        out[b, order[:NQ], :] = res.results[c]["out_fm"].T
    return out


# revision 27
# speedup vs baseline: 1.0398x; 1.0398x over previous
"""MoIE transformer block — Bass/Tile kernel for 8 Trainium2 NeuronCores.

Contract: kernel(**inputs) takes FULL unsharded inputs (B=4, S=2048, D=1024,
fp32) and returns the FULL [4, 2048, 1024] fp32 output.

Sharding: data-parallel, 2 cores per batch (core c -> batch c//2, half c%2).
Each core owns 1024 query rows and recomputes LN + k/v branches over the full
2048-row sequence of its batch (weights replicated). The SPMD program is
identical on all cores; role differences (which half) are pure data:

  - the host feeds x[b]^T with COLUMNS PERMUTED so the core's own query
    tokens always sit in columns [0:1024),
  - causality is applied with a host-built 0/1 mask (multiplied into exp(s)),
    so the attention loop shape is uniform across cores.

Layout is feature-major ([d partitions, tokens free]) end to end:
  - LN stats via ones-vector matmuls (sum / sum-of-squares over partitions),
    rstd/offset broadcast back across partitions with a K=1 ones matmul.
  - branch matmuls: out[o_tile, n] = W^T_tile.T @ ln1_tile, bf16 in / fp32
    PSUM out; routing chain = Silu(comp+b) on ACT, (match/32 - cost) and
    relu on DVE, passthrough select via copy_predicated with mrl as the
    predicate (mrl != 0  <=>  rl > 0).
  - attention computes scores TRANSPOSED ([k partitions, q free], K
    stationary) so softmax needs no P-transpose: exp on ACT (constant -8
    bias for range safety; exact softmax is shift-invariant), 0/1 mask
    multiply, denominator via ones-matmul over partitions, P@V directly
    from the transposed exp tiles. Only V needs a real transpose (DMA xbar,
    SBUF->SBUF, 128x128 bf16 blocks), streamed chunk by chunk.
  - o-branch + residual in fp32, output stored feature-major; host
    transposes back.
"""

import contextlib
import os
import sys

import numpy as np

for _p in ("/opt/trn_rl_repo",):
    if _p not in sys.path and os.path.isdir(_p):
        sys.path.insert(0, _p)

import ml_dtypes  # noqa: E402

import concourse.bass as bass  # noqa: E402,F401
import concourse.mybir as mybir  # noqa: E402
import concourse.tile as tile  # noqa: E402
from concourse import bacc  # noqa: E402
from concourse.bass_utils import run_bass_kernel_spmd  # noqa: E402

B, S, D = 4, 2048, 1024
P = 128
DC = D // P          # 8 d-chunks
KB = S // P          # 16 key blocks
NQ = S // 2          # 1024 query tokens per core
EPS_LN = 1e-5
EPS_MAS = 1e-9
INV_SQRT_D = 1.0 / 32.0
EXP_SHIFT = -8.0     # constant softmax shift (exact softmax is shift-invariant)

F32 = mybir.dt.float32
BF16 = mybir.dt.bfloat16
OP = mybir.AluOpType
ACTF = mybir.ActivationFunctionType

BF = ml_dtypes.bfloat16

USE_COLLECTIVES = bool(int(os.environ.get("BASS_USE_CC", "1")))

_COMPILED = {}
LAST_EXEC_TIME_NS = None

WEIGHT_NAMES = [f"{n}_{w}" for n in "qkvo" for w in ("mu", "pr")]
# vecs rows: ln_g, ln_b, cost_q, cost_k, cost_v, cost_o, bias_q, bias_k, bias_v, bias_o
VROW = {"ln_g": 0, "ln_b": 1, "cost_q": 2, "cost_k": 3, "cost_v": 4,
        "cost_o": 5, "bias_q": 6, "bias_k": 7, "bias_v": 8, "bias_o": 9}


def _branch(nc, psum, sbuf, n, rhs_sb, wmu, wpr, negcost_ap, cost32_ap,
            bias_ap, pass_sb, out_cb):
    """Routed-expert branch, feature-major.

    rhs_sb/pass_sb: [P, DC, n] bf16.  wmu/wpr: [P, DC, D] bf16 (lhsT layout).
    negcost_ap/cost32_ap/bias_ap: [P, DC] fp32 per-partition scalar views
    (-cost, 32*cost, mu bias).
    out_cb(ot, j) -> (dst_ap [P,512], finish_fn|None): routed bf16 output
    chunk destination; finish_fn runs after the chunk is complete.

    out = comp*relu(rl) + pass*(rl<=0), rl = match/32 - cost:
      ACT: comp = Silu(cp + bias);  mrl = Relu(mp/32 - cost)  [fused scale+bias]
      DVE: passc = (mp <= 32cost)*pass; prod = comp*mrl; dst = prod + passc
    """
    nch = n // 512
    for ot in range(DC):
        for j in range(nch):
            cp = psum.tile([P, 512], F32, tag="mm")
            mp = psum.tile([P, 512], F32, tag="mm")
            for dc in range(DC):
                nc.tensor.matmul(cp, wmu[:, dc, ot * P:(ot + 1) * P],
                                 rhs_sb[:, dc, j * 512:(j + 1) * 512],
                                 start=(dc == 0), stop=(dc == DC - 1))
            for dc in range(DC):
                nc.tensor.matmul(mp, wpr[:, dc, ot * P:(ot + 1) * P],
                                 rhs_sb[:, dc, j * 512:(j + 1) * 512],
                                 start=(dc == 0), stop=(dc == DC - 1))
            comp = sbuf.tile([P, 512], BF16, tag="comp")
            nc.scalar.activation(comp, cp, ACTF.Silu, bias=bias_ap[:, ot:ot + 1])
            mrl = sbuf.tile([P, 512], BF16, tag="mrl")
            nc.scalar.activation(mrl, mp, ACTF.Relu,
                                 bias=negcost_ap[:, ot:ot + 1], scale=INV_SQRT_D)
            passc = sbuf.tile([P, 512], BF16, tag="passc")
            nc.vector.scalar_tensor_tensor(
                passc, mp, cost32_ap[:, ot:ot + 1],
                pass_sb[:, ot, j * 512:(j + 1) * 512], OP.is_le, OP.mult)
            prod = sbuf.tile([P, 512], BF16, tag="prod")
            nc.vector.tensor_mul(prod, comp, mrl)
            dst, finish = out_cb(ot, j)
            nc.vector.tensor_add(dst, prod, passc)
            if finish is not None:
                finish()


def _build_program():
    nc = bacc.Bacc("TRN2", target_bir_lowering=False, debug=False,
                   enable_asserts=False, num_devices=8)

    NKV = NQ if USE_COLLECTIVES else S   # kv tokens computed locally per core

    xT = nc.dram_tensor("xT", [D, S], BF16, kind="ExternalInput").ap()
    maskT = nc.dram_tensor("maskT", [S, NQ], BF16, kind="ExternalInput").ap()
    vecs = nc.dram_tensor("vecs", [10, D], F32, kind="ExternalInput").ap()
    w_dram = {n: nc.dram_tensor(f"w_{n}", [D, D], BF16, kind="ExternalInput").ap()
              for n in WEIGHT_NAMES}
    out = nc.dram_tensor("out_fm", [D, NQ], F32, kind="ExternalOutput").ap()
    attn_dram = nc.dram_tensor("attn_bounce", [D, NQ], BF16).ap()
    k_loc = nc.dram_tensor("k_loc", [D, NKV], BF16).ap()
    v_loc = nc.dram_tensor("v_loc", [D, NKV], BF16).ap()
    if USE_COLLECTIVES:
        k_gath = nc.dram_tensor("k_gath", [2, D, NQ], BF16).ap()
        v_gath = nc.dram_tensor("v_gath", [2, D, NQ], BF16).ap()

    xT3 = xT.rearrange("(dc p) n -> p dc n", p=P)
    maskT3 = maskT.rearrange("(kb p) q -> p kb q", p=P)
    vecs3 = vecs.rearrange("r (oc p) -> p r oc", p=P)
    out3 = out.rearrange("(dc p) n -> p dc n", p=P)
    attn3 = attn_dram.rearrange("(dc p) n -> p dc n", p=P)
    k_loc3 = k_loc.rearrange("(dc p) n -> p dc n", p=P)
    v_loc3 = v_loc.rearrange("(dc p) n -> p dc n", p=P)

    with tile.TileContext(nc) as tc, contextlib.ExitStack() as ctx:
        consts = ctx.enter_context(tc.tile_pool(name="consts", bufs=1))
        wpool = ctx.enter_context(tc.tile_pool(name="weights", bufs=4))
        stream = ctx.enter_context(tc.tile_pool(name="stream", bufs=3))
        psum = ctx.enter_context(tc.tile_pool(name="psum", bufs=6, space="PSUM"))
        psum1 = ctx.enter_context(tc.tile_pool(name="psum1", bufs=2, space="PSUM"))

        vec_sb = consts.tile([P, 10, DC], F32)
        nc.sync.dma_start(vec_sb, vecs3)
        ones_bf = consts.tile([P, 1], BF16)
        nc.vector.memset(ones_bf, 1.0)
        ones_row = consts.tile([1, P], F32)
        nc.vector.memset(ones_row, 1.0)
        eps_ap = consts.tile([P, 1], F32)
        nc.vector.memset(eps_ap, float(EPS_LN))
        esh_ap = consts.tile([P, 1], F32)
        nc.vector.memset(esh_ap, float(EXP_SHIFT))
        negcost = consts.tile([P, 4, DC], F32)
        cost32 = consts.tile([P, 4, DC], F32)
        for i, nme in enumerate("qkvo"):
            nc.vector.tensor_scalar_mul(negcost[:, i, :],
                                        vec_sb[:, VROW[f"cost_{nme}"], :], -1.0)
            nc.vector.tensor_scalar_mul(cost32[:, i, :],
                                        vec_sb[:, VROW[f"cost_{nme}"], :], 32.0)
        CIDX = {n: i for i, n in enumerate("qkvo")}

        def vrow(name):
            return vec_sb[:, VROW[name], :]

        def wload(name):
            t = wpool.tile([P, DC, D], BF16, tag="w")
            nc.sync.dma_start(t, w_dram[name].rearrange("(dc p) o -> p dc o", p=P))
            return t

        q_pool = ctx.enter_context(tc.tile_pool(name="q_pool", bufs=1))
        q_fm = q_pool.tile([P, DC, NQ], BF16, tag="q_fm")

        with tc.tile_pool(name="ln_scope", bufs=1) as lnp:
            ln1 = lnp.tile([P, DC, NKV], BF16, tag="ln1")

            # ------------- LayerNorm (feature-major, local tokens) -------------
            with tc.tile_pool(name="ln_bc", bufs=1) as lnbc, \
                 tc.tile_pool(name="ln_stream", bufs=16) as lns, \
                 tc.tile_pool(name="ln_stats", bufs=6) as lnst, \
                 tc.tile_pool(name="ln_sq", bufs=8) as lnsq:
                rstd_bc = lnbc.tile([P, NKV], BF16, tag="rstd_bc")
                boff_bc = lnbc.tile([P, NKV], BF16, tag="boff_bc")
                xts = {}
                for j in range(NKV // 512):
                    s_ps = psum1.tile([1, 512], F32, tag="vsum")
                    q_ps = psum1.tile([1, 512], F32, tag="vsum")
                    sqs = []
                    for dc in range(DC):
                        xt = lns.tile([P, 512], BF16, tag="xt")
                        nc.sync.dma_start(xt, xT3[:, dc, j * 512:(j + 1) * 512])
                        xts[(j, dc)] = xt
                        sq = lnsq.tile([P, 512], BF16, tag="sq")
                        nc.scalar.square(sq, xt)
                        sqs.append(sq)
                        nc.tensor.matmul(s_ps, ones_bf, xt,
                                         start=(dc == 0), stop=(dc == DC - 1))
                    for dc in range(DC):
                        nc.tensor.matmul(q_ps, ones_bf, sqs[dc],
                                         start=(dc == 0), stop=(dc == DC - 1))
                    mean = lnst.tile([1, 512], F32, tag="stat")
                    nc.vector.tensor_scalar_mul(mean, s_ps, 1.0 / D)
                    var = lnst.tile([1, 512], F32, tag="stat")
                    nc.vector.tensor_scalar_mul(var, q_ps, 1.0 / D)
                    m2 = lnst.tile([1, 512], F32, tag="stat")
                    nc.vector.tensor_mul(m2, mean, mean)
                    nc.vector.tensor_sub(var, var, m2)
                    nc.scalar.activation(var, var, ACTF.Sqrt, bias=eps_ap[:1])
                    rstd = lnst.tile([1, 512], F32, tag="stat")
                    nc.vector.reciprocal(rstd, var)
                    boff = lnst.tile([1, 512], F32, tag="stat")
                    nc.vector.tensor_mul(boff, mean, rstd)
                    nc.vector.tensor_scalar_mul(boff, boff, -1.0)
                    # broadcast across partitions via K=1 ones matmul
                    bc = psum1.tile([P, 512], F32, tag="vsum")
                    nc.tensor.matmul(bc, ones_row, rstd, start=True, stop=True)
                    nc.scalar.copy(rstd_bc[:, j * 512:(j + 1) * 512], bc)
                    bc2 = psum1.tile([P, 512], F32, tag="vsum")
                    nc.tensor.matmul(bc2, ones_row, boff, start=True, stop=True)
                    nc.scalar.copy(boff_bc[:, j * 512:(j + 1) * 512], bc2)
                # pass 2: apply from the resident pass-1 tiles, j-chunked
                # so the k-branch can start on chunk 0 early
                for j in range(NKV // 512):
                    sl = slice(j * 512, (j + 1) * 512)
                    for dc in range(DC):
                        dst = ln1[:, dc, sl]
                        nc.vector.tensor_mul(dst, xts[(j, dc)], rstd_bc[:, sl])
                        nc.vector.tensor_add(dst, dst, boff_bc[:, sl])
                        nc.vector.tensor_scalar(dst, dst,
                                                vrow("ln_g")[:, dc:dc + 1],
                                                vrow("ln_b")[:, dc:dc + 1],
                                                OP.mult, OP.add)

            # ------------- k / v / q branches -------------
            # k and v stream their routed output chunks straight to DRAM
            # (k_loc / v_loc), to be (all-)gathered + reloaded for attention.
            def dram_out(dram3):
                def cb(ot, j):
                    chunk = stream.tile([P, 512], BF16, tag="dchunk")

                    def finish():
                        nc.sync.dma_start(dram3[:, ot, j * 512:(j + 1) * 512],
                                          chunk)
                    return chunk, finish
                return cb

            wkm, wkp = wload("k_mu"), wload("k_pr")
            _branch(nc, psum, stream, NKV, ln1, wkm, wkp,
                    negcost[:, CIDX["k"], :], cost32[:, CIDX["k"], :],
                    vrow("bias_k"), ln1, dram_out(k_loc3))
            wvm, wvp = wload("v_mu"), wload("v_pr")
            _branch(nc, psum, stream, NKV, ln1, wvm, wvp,
                    negcost[:, CIDX["v"], :], cost32[:, CIDX["v"], :],
                    vrow("bias_v"), ln1, dram_out(v_loc3))
            if USE_COLLECTIVES:
                groups = [[0, 1], [2, 3], [4, 5], [6, 7]]
                nc.gpsimd.collective_compute(
                    "AllGather", mybir.AluOpType.bypass, replica_groups=groups,
                    ins=[k_loc[:]], outs=[k_gath[:]])
                nc.gpsimd.collective_compute(
                    "AllGather", mybir.AluOpType.bypass, replica_groups=groups,
                    ins=[v_loc[:]], outs=[v_gath[:]])
            wqm, wqp = wload("q_mu"), wload("q_pr")
            _branch(nc, psum, stream, NQ, ln1[:, :, :NQ], wqm, wqp,
                    negcost[:, CIDX["q"], :], cost32[:, CIDX["q"], :],
                    vrow("bias_q"), ln1[:, :, :NQ],
                    lambda ot, j: (q_fm[:, ot, j * 512:(j + 1) * 512], None))

        # ---------------- attention ----------------
        with tc.tile_pool(name="kv_pool", bufs=1) as kvp, \
             tc.tile_pool(name="attn_tmp", bufs=1) as at, \
             tc.tile_pool(name="mask_pool", bufs=1) as mkp, \
             tc.tile_pool(name="attn_sc", bufs=2) as asc, \
             tc.tile_pool(name="attn_st", bufs=3) as ast:
            k_fm = kvp.tile([P, DC, S], BF16, tag="k_fm")
            v_tm = kvp.tile([P, KB, D], BF16, tag="v_tm")
            if USE_COLLECTIVES:
                for g in range(2):
                    kg3 = k_gath[g].rearrange("(dc p) n -> p dc n", p=P)
                    for dc in range(DC):
                        nc.sync.dma_start(k_fm[:, dc, g * NQ:(g + 1) * NQ],
                                          kg3[:, dc, :])
                    for dt in range(DC):
                        nc.sync.dma_start(
                            v_tm[:, g * 8:(g + 1) * 8, dt * P:(dt + 1) * P],
                            v_gath[g, dt * P:(dt + 1) * P, :],
                            transpose=True)
            else:
                for dc in range(DC):
                    nc.sync.dma_start(k_fm[:, dc, :], k_loc3[:, dc, :])
                for dt in range(DC):
                    nc.sync.dma_start(
                        v_tm[:, :, dt * P:(dt + 1) * P],
                        v_loc[dt * P:(dt + 1) * P, :],
                        transpose=True)

            for qg in range(NQ // 512):
                q0 = qg * 512
                # Interleaved query-block assignment (h=0 even 128-blocks,
                # h=1 odd): for query group 0 (own blocks 0..3 -> max token
                # 1023), key blocks {4..7, 12..15} hold tokens >= 1024 on
                # BOTH core roles -> fully masked -> statically skipped.
                kbs = [0, 1, 2, 3, 8, 9, 10, 11] if qg == 0 else list(range(KB))
                mask_sb = mkp.tile([P, KB, 512], BF16, tag="mask")
                nc.sync.dma_start(mask_sb, maskT3[:, :, q0:q0 + 512])
                expT = at.tile([P, KB, 512], BF16, tag="expT")
                for kb in kbs:
                    sT = psum.tile([P, 512], F32, tag="mm")
                    for dc in range(DC):
                        nc.tensor.matmul(
                            sT, k_fm[:, dc, kb * P:(kb + 1) * P],
                            q_fm[:, dc, q0:q0 + 512],
                            start=(dc == 0), stop=(dc == DC - 1))
                    # exp(s/sqrt(D) - 8); masked cols zeroed after
                    nc.scalar.activation(expT[:, kb, :], sT, ACTF.Exp,
                                         bias=esh_ap, scale=INV_SQRT_D)
                    nc.vector.tensor_mul(expT[:, kb, :], expT[:, kb, :],
                                         mask_sb[:, kb, :])
                sum_ps = psum1.tile([1, 512], F32, tag="vsum")
                for i, kb in enumerate(kbs):
                    nc.tensor.matmul(sum_ps, ones_bf, expT[:, kb, :],
                                     start=(i == 0), stop=(i == len(kbs) - 1))
                rcp = asc.tile([1, 512], F32, tag="rcp")
                nc.vector.reciprocal(rcp, sum_ps)
                bc = psum1.tile([P, 512], F32, tag="vsum")
                nc.tensor.matmul(bc, ones_row, rcp, start=True, stop=True)
                rcpb = asc.tile([P, 512], F32, tag="rcpb")
                nc.scalar.copy(rcpb, bc)
                for dt in range(DC):
                    av = psum.tile([P, 512], F32, tag="mm")
                    for i, kb in enumerate(kbs):
                        nc.tensor.matmul(av, v_tm[:, kb, dt * P:(dt + 1) * P],
                                         expT[:, kb, :],
                                         start=(i == 0), stop=(i == len(kbs) - 1))
                    a_sb = ast.tile([P, 512], BF16, tag="a_sb")
                    nc.vector.tensor_mul(a_sb, av, rcpb)
                    nc.sync.dma_start(attn3[:, dt, q0:q0 + 512], a_sb)

        # ---------------- o branch + residual ----------------
        with tc.tile_pool(name="xq_pool", bufs=1) as xqp, \
             tc.tile_pool(name="o_stream", bufs=3) as ost:
            xq = xqp.tile([P, DC, NQ], BF16, tag="xq")
            nc.sync.dma_start(xq, xT3[:, :, :NQ])
            attn = xqp.tile([P, DC, NQ], BF16, tag="attn_sb")
            for j0 in range(NQ // 512):
                for dc in range(DC):
                    nc.sync.dma_start(attn[:, dc, j0 * 512:(j0 + 1) * 512],
                                      attn3[:, dc, j0 * 512:(j0 + 1) * 512])
            wom, wop = wload("o_mu"), wload("o_pr")

            nch = NQ // 512
            for j in range(nch):
                for ot in range(DC):
                    cp = psum.tile([P, 512], F32, tag="mm")
                    mp = psum.tile([P, 512], F32, tag="mm")
                    for dc in range(DC):
                        nc.tensor.matmul(cp, wom[:, dc, ot * P:(ot + 1) * P],
                                         attn[:, dc, j * 512:(j + 1) * 512],
                                         start=(dc == 0), stop=(dc == DC - 1))
                    for dc in range(DC):
                        nc.tensor.matmul(mp, wop[:, dc, ot * P:(ot + 1) * P],
                                         attn[:, dc, j * 512:(j + 1) * 512],
                                         start=(dc == 0), stop=(dc == DC - 1))
                    comp = stream.tile([P, 512], BF16, tag="comp")
                    nc.scalar.activation(comp, cp, ACTF.Silu,
                                         bias=vrow("bias_o")[:, ot:ot + 1])
                    mrl = stream.tile([P, 512], BF16, tag="mrl")
                    nc.scalar.activation(mrl, mp, ACTF.Relu,
                                         bias=negcost[:, CIDX["o"], ot:ot + 1],
                                         scale=INV_SQRT_D)
                    passc = ost.tile([P, 512], F32, tag="passcf")
                    nc.vector.scalar_tensor_tensor(
                        passc, mp, cost32[:, CIDX["o"], ot:ot + 1],
                        attn[:, ot, j * 512:(j + 1) * 512], OP.is_le, OP.mult)
                    prod = ost.tile([P, 512], F32, tag="prodf")
                    nc.vector.tensor_mul(prod, comp, mrl)
                    ot_f = ost.tile([P, 512], F32, tag="outf")
                    nc.vector.tensor_add(ot_f, prod, passc)
                    nc.vector.tensor_add(ot_f, ot_f,
                                         xq[:, ot, j * 512:(j + 1) * 512])
                    nc.sync.dma_start(out3[:, ot, j * 512:(j + 1) * 512], ot_f)

    nc.compile()
    return nc


def _get_program():
    if "nc" not in _COMPILED:
        _COMPILED["nc"] = _build_program()
    return _COMPILED["nc"]


def _prepare_inputs(x, ln_g, ln_b,
                    q_mu_w, q_mu_b, q_proto, q_gate,
                    k_mu_w, k_mu_b, k_proto, k_gate,
                    v_mu_w, v_mu_b, v_proto, v_gate,
                    o_mu_w, o_mu_b, o_proto, o_gate):
    x = np.asarray(x, dtype=np.float32)

    w_host = {}
    for n, (mu_w, proto) in zip("qkvo", [(q_mu_w, q_proto), (k_mu_w, k_proto),
                                         (v_mu_w, v_proto), (o_mu_w, o_proto)]):
        w_host[f"w_{n}_mu"] = np.ascontiguousarray(np.asarray(mu_w).T).astype(BF)
        w_host[f"w_{n}_pr"] = np.ascontiguousarray(np.asarray(proto).T).astype(BF)

    vecs = np.zeros((10, D), np.float32)
    vecs[VROW["ln_g"]] = np.asarray(ln_g)
    vecs[VROW["ln_b"]] = np.asarray(ln_b)
    for n, gate, bias in [("q", q_gate, q_mu_b), ("k", k_gate, k_mu_b),
                          ("v", v_gate, v_mu_b), ("o", o_gate, o_mu_b)]:
        g = np.asarray(gate, np.float32)
        vecs[VROW[f"cost_{n}"]] = g / (np.abs(g).max() + EPS_MAS)
        vecs[VROW[f"bias_{n}"]] = np.asarray(bias, np.float32)

    blk = np.arange(P)

    def role_order(h):
        """Interleaved 128-blocks: h=0 -> even blocks, h=1 -> odd; own query
        tokens first, complement after (complement unused on the CC path)."""
        own = ((np.arange(S // 256) * 2 + h)[:, None] * P + blk).ravel()
        other = ((np.arange(S // 256) * 2 + (1 - h))[:, None] * P + blk).ravel()
        return np.concatenate([own, other])

    in_maps = []
    orders = []
    for c in range(8):
        b, h = c // 2, c % 2
        order = role_order(h)
        orders.append((b, order))
        xTc = np.ascontiguousarray(x[b][order].T).astype(BF)
        if USE_COLLECTIVES:
            key_tok = np.concatenate([role_order(0)[:NQ], role_order(1)[:NQ]])
        else:
            key_tok = order
        maskT = (key_tok[:, None] <= order[None, :NQ]).astype(BF)
        m = dict(w_host)
        m["xT"] = xTc
        m["maskT"] = np.ascontiguousarray(maskT)
        m["vecs"] = vecs
        in_maps.append(m)
    return in_maps, orders


def _ensure_ntff_hook():
    """Install the axon NTFF profile hook if the image's antenv lacks it."""
    try:
        from antenv.axon_hooks import get_axon_ntff_profile_hook  # noqa: F401
        return
    except ImportError:
        pass
    try:
        import types

        from trn_agent_boot.trn_boot import _ntff_profile_via_ctypes
        hook = _ntff_profile_via_ctypes("/opt/axon/libaxon_pjrt.so")
        mod = types.ModuleType("antenv.axon_hooks")
        mod.get_axon_ntff_profile_hook = lambda: hook
        mod.set_axon_ntff_profile_hook = lambda h: None
        import antenv
        sys.modules["antenv.axon_hooks"] = mod
        antenv.axon_hooks = mod
    except Exception as e:  # degrade to untraced run
        print(f"ntff hook install failed: {e}", file=sys.stderr)


def kernel(**inputs):
    global LAST_EXEC_TIME_NS
    in_maps, orders = _prepare_inputs(**inputs)
    nc = _get_program()
    trace = bool(int(os.environ.get("BASS_KERNEL_TRACE", "0")))
    if trace:
        _ensure_ntff_hook()
    tdir = os.environ.get("BASS_KERNEL_TRACE_DIR") or None
    res = run_bass_kernel_spmd(nc, in_maps, list(range(8)), trace=trace,
                               tmpdir=tdir)
    LAST_EXEC_TIME_NS = res.exec_time_ns

    out = np.empty((B, S, D), np.float32)
    for c in range(8):
        b, order = orders[c]
        out[b, order[:NQ], :] = res.results[c]["out_fm"].T
    return out
